# revision 3
# baseline (speedup 1.0000x reference)
"""DLinear fused kernel for 8 TRN2 NeuronCores.

Math: the whole module is linear in x.
  trend = x @ A^T (A = edge-padded moving-average matrix, window 25)
  out[b,n,:] = sum_c wf_c * ( x[b,c,n,:] @ (Ws + (Wt-Ws)@A)^T ) + bias
  bias = sum(wf) * (bs + bt) + bf

Host precomputes the tiny effective weight Weff = Ws + (Wt-Ws)@A in f64
(weights only). Device per core (8 batches):
  - channel combine xc' = (x_a*r_a + x_b)*r_b + x_c  (2 fused STT ops,
    bf16; lc chunks 0-2 on DVE, chunk 3 on GPSIMD) with channels sorted
    by |wf| ascending, r_a = wf_a/wf_b, r_b = wf_b/wf_c; the final scale
    wf_c is folded into the weights.
  - out chunks [128bn, 336]: K=1 bias matmul + 4 accumulated matmuls
    over l-chunks (lhsT = xc chunk [128l, 128bn], rhs = W^T [128l, 336])
  - PSUM drained by plain ScalarE copy (DVE stays free for the combine)
Input DMA: one 768KB transfer per (bb, lc) with 6KB-contiguous rows
([l, c, bn] free-dim layout prepared on host).
"""

import numpy as np
import ml_dtypes

import concourse.bacc as bacc
import concourse.mybir as mybir
import concourse.tile as tile
from concourse.bass_utils import run_bass_kernel_spmd

N_CORES = 8
B, C, N, L, P = 64, 3, 512, 512, 336
KERNEL_W, PAD = 25, 12
BPC = B // N_CORES          # batches per core = 8
BN = BPC * N                # rows per core = 4096
BB, BNB = 4, 1024           # bn blocks per core, rows per block
LC = 4                      # l chunks of 128
NJ = BNB // 128             # 128-row chunks per bn block = 8

BF16 = mybir.dt.bfloat16
F32 = mybir.dt.float32

LAST_RESULT = None
_CACHE = {}


def _movavg_matrix():
    A = np.zeros((L, L), np.float64)
    for lp in range(L):
        for kk in range(lp - PAD, lp + PAD + 1):
            A[lp, min(max(kk, 0), L - 1)] += 1.0 / KERNEL_W
    return A


def _build(r_a, r_b):
    nc = bacc.Bacc("TRN2", target_bir_lowering=False, debug=False)
    x_d = nc.dram_tensor("x", (BB, LC, 128, C * BNB), BF16, kind="ExternalInput")
    w_d = nc.dram_tensor("w", (LC, 128, P), BF16, kind="ExternalInput")
    b_d = nc.dram_tensor("bias", (1, P), BF16, kind="ExternalInput")
    o_d = nc.dram_tensor("o", (BB, 128, NJ * P), F32, kind="ExternalOutput")

    with tile.TileContext(nc) as tc:
        with (
            tc.tile_pool(name="const", bufs=1) as constp,
            tc.tile_pool(name="xin", bufs=3) as xinp,
            tc.tile_pool(name="xcp", bufs=2) as xcp,
            tc.tile_pool(name="ps", bufs=4, space="PSUM") as psp,
            tc.tile_pool(name="ostage", bufs=2) as osp,
        ):
            wts = []
            for k in range(LC):
                wt = constp.tile([128, P], BF16, tag=f"w{k}", name=f"w{k}")
                nc.sync.dma_start(wt[:], w_d[k])
                wts.append(wt)
            btile = constp.tile([1, P], BF16, tag="bias", name="bias")
            nc.sync.dma_start(btile[:], b_d[:])
            ones = constp.tile([1, 128], BF16, tag="ones", name="ones")
            nc.gpsimd.memset(ones[:], 1.0)

            for bb in range(BB):
                xcs = []
                for lc in range(LC):
                    xf = xinp.tile([128, C * BNB], BF16, tag=f"x{lc}",
                                   name=f"x{lc}_{bb}")
                    nc.sync.dma_start(xf[:], x_d[bb, lc])
                    xa = xf[:, 0:BNB]
                    xb = xf[:, BNB:2 * BNB]
                    xk = xf[:, 2 * BNB:3 * BNB]
                    eng = nc.vector  # STT is DVE-only on TRN2 (Pool lacks the opcode)
                    t = xcp.tile([128, BNB], BF16, tag=f"t{lc}", name=f"t{lc}_{bb}")
                    eng.scalar_tensor_tensor(
                        t[:], xa, float(r_a), xb,
                        mybir.AluOpType.mult, mybir.AluOpType.add,
                    )
                    xc = xcp.tile([128, BNB], BF16, tag=f"xc{lc}", name=f"xc{lc}_{bb}")
                    eng.scalar_tensor_tensor(
                        xc[:], t[:], float(r_b), xk,
                        mybir.AluOpType.mult, mybir.AluOpType.add,
                    )
                    xcs.append(xc)

                ost = osp.tile([128, NJ * P], F32, tag="ost", name=f"ost{bb}")
                for j in range(NJ):
                    ps = psp.tile([128, P], F32, tag="ps", name=f"ps{bb}_{j}")
                    nc.tensor.matmul(ps[:], ones[:], btile[:],
                                     start=True, stop=False)
                    for k in range(LC):
                        nc.tensor.matmul(
                            ps[:],
                            xcs[k][:, j * 128:(j + 1) * 128],
                            wts[k][:],
                            start=False,
                            stop=(k == LC - 1),
                        )
                    nc.scalar.copy(ost[:, j * P:(j + 1) * P], ps[:])
                nc.sync.dma_start(o_d[bb], ost[:])

    nc.compile()
    return nc


def kernel(x, Ws, bs, Wt, bt, Wf, bf):
    global LAST_RESULT
    # ---- host-side weight folding (f64, weights only) ----
    A = _movavg_matrix()
    Weff = Ws.astype(np.float64) + (Wt.astype(np.float64) - Ws.astype(np.float64)) @ A
    wf = Wf[0].astype(np.float64)                      # (3,)
    order = np.argsort(np.abs(wf))                     # ascending |wf|
    ca, cb, cc = int(order[0]), int(order[1]), int(order[2])
    r_a = float(wf[ca] / wf[cb]) if wf[cb] != 0 else 0.0
    r_b = float(wf[cb] / wf[cc]) if wf[cc] != 0 else 0.0
    s = float(wf[cc])
    Wp = (s * Weff) if s != 0 else Weff * 0.0          # (336, 512)
    WT = np.ascontiguousarray(Wp.T).reshape(LC, 128, P).astype(ml_dtypes.bfloat16)
    bias = wf.sum() * (bs.astype(np.float64) + bt.astype(np.float64)) + float(bf[0])
    bias_b = bias.astype(ml_dtypes.bfloat16)[None, :]  # (1, 336)

    # ---- build / compile (cached per (r_a, r_b)) ----
    key = (round(r_a, 12), round(r_b, 12))
    if key not in _CACHE:
        _CACHE[key] = _build(r_a, r_b)
    nc = _CACHE[key]

    # ---- host-side sharding / layout (pure data movement) ----
    xb16 = x.astype(ml_dtypes.bfloat16)                # (64,3,512,512)
    xr = xb16.reshape(N_CORES, BPC, C, N, L)
    xr = xr.transpose(0, 2, 4, 1, 3)                   # [core, c, l, bl, n]
    xr = xr[:, (ca, cb, cc)]                           # channel order by |wf|
    xr = xr.reshape(N_CORES, C, LC, 128, BB, BNB)
    xr = xr.transpose(0, 4, 2, 3, 1, 5)                # [core, bb, lc, 128, c, bn]
    xr = xr.reshape(N_CORES, BB, LC, 128, C * BNB)

    in_maps = []
    for i in range(N_CORES):
        in_maps.append({
            "x": np.ascontiguousarray(xr[i]),
            "w": WT,
            "bias": bias_b,
        })

    res = run_bass_kernel_spmd(nc, in_maps, core_ids=list(range(N_CORES)))
    LAST_RESULT = res

    # ---- gather / unshard ----
    outs = []
    for i in range(N_CORES):
        o = res.results[i]["o"]                        # (4, 128, 2688) f32
        o = o.reshape(BB, 128, NJ, P).transpose(0, 2, 1, 3).reshape(BPC, N, P)
        outs.append(o)
    out = np.stack(outs).reshape(B, N, P)[:, None]     # (64, 1, 512, 336)
    return out.astype(np.float32)


# revision 5
# speedup vs baseline: 1.1317x; 1.1317x over previous
"""DLinear fused kernel for 8 TRN2 NeuronCores.

Math: the whole module is linear in x.
  trend = x @ A^T (A = edge-padded moving-average matrix, window 25)
  out[b,n,:] = sum_c wf_c * ( x[b,c,n,:] @ (Ws + (Wt-Ws)@A)^T ) + bias
  bias = sum(wf) * (bs + bt) + bf

Host precomputes the tiny effective weight Weff = Ws + (Wt-Ws)@A in f64
(weights only). Device per core (8 batches):
  - channel combine xc' = (x_a*r_a + x_b)*r_b + x_c  (2 fused DVE STT ops,
    bf16) with channels sorted by |wf| ascending, r_a = wf_a/wf_b,
    r_b = wf_b/wf_c; the final scale wf_c is folded into the weights.
  - matmul weights-stationary: out[112p, 512bn] += WT[k][:,pc].T @ xc[k]
    accumulated over 4 l-chunks; N=512 streams, stationary reused.
  - PSUM drain on ScalarE with fused per-partition bias add.
Input DMA: one 768KB transfer per (bb, lc) with 6KB-contiguous rows
([l, c, bn] free-dim layout prepared on host).
"""

import numpy as np
import ml_dtypes

import concourse.bacc as bacc
import concourse.mybir as mybir
import concourse.tile as tile
from concourse.bass_utils import run_bass_kernel_spmd

N_CORES = 8
B, C, N, L, P = 64, 3, 512, 512, 336
KERNEL_W, PAD = 25, 12
BPC = B // N_CORES          # batches per core = 8
BN = BPC * N                # rows per core = 4096
BB, BNB = 4, 1024           # bn blocks per core, rows per block
LC = 4                      # l chunks of 128
PC, PCW = 3, 112            # p chunks x width (3*112 = 336)
NT, NTW = 2, 512            # bn tiles per block x width

BF16 = mybir.dt.bfloat16
F32 = mybir.dt.float32

LAST_RESULT = None
_CACHE = {}


def _movavg_matrix():
    A = np.zeros((L, L), np.float64)
    for lp in range(L):
        for kk in range(lp - PAD, lp + PAD + 1):
            A[lp, min(max(kk, 0), L - 1)] += 1.0 / KERNEL_W
    return A


def _build(r_a, r_b):
    nc = bacc.Bacc("TRN2", target_bir_lowering=False, debug=False)
    x_d = nc.dram_tensor("x", (BB, LC, 128, C * BNB), BF16, kind="ExternalInput")
    w_d = nc.dram_tensor("w", (LC, 128, P), BF16, kind="ExternalInput")
    b_d = nc.dram_tensor("bias", (PCW, PC), F32, kind="ExternalInput")
    o_d = nc.dram_tensor("o", (BB, PCW, PC * BNB), F32, kind="ExternalOutput")

    with tile.TileContext(nc) as tc:
        with (
            tc.tile_pool(name="const", bufs=1) as constp,
            tc.tile_pool(name="xin", bufs=3) as xinp,
            tc.tile_pool(name="xcp", bufs=2) as xcp,
            tc.tile_pool(name="ps", bufs=6, space="PSUM") as psp,
            tc.tile_pool(name="ostage", bufs=2) as osp,
        ):
            wts = []
            for k in range(LC):
                wt = constp.tile([128, P], BF16, tag=f"w{k}", name=f"w{k}")
                nc.sync.dma_start(wt[:], w_d[k])
                wts.append(wt)
            btile = constp.tile([PCW, PC], F32, tag="bias", name="bias")
            nc.sync.dma_start(btile[:], b_d[:])

            for bb in range(BB):
                xcs = []
                for lc in range(LC):
                    xf = xinp.tile([128, C * BNB], BF16, tag=f"x{lc}",
                                   name=f"x{lc}_{bb}")
                    nc.sync.dma_start(xf[:], x_d[bb, lc])
                    xa = xf[:, 0:BNB]
                    xb = xf[:, BNB:2 * BNB]
                    xk = xf[:, 2 * BNB:3 * BNB]
                    t = xcp.tile([128, BNB], BF16, tag=f"t{lc}", name=f"t{lc}_{bb}")
                    nc.vector.scalar_tensor_tensor(
                        t[:], xa, float(r_a), xb,
                        mybir.AluOpType.mult, mybir.AluOpType.add,
                    )
                    xc = xcp.tile([128, BNB], BF16, tag=f"xc{lc}", name=f"xc{lc}_{bb}")
                    nc.vector.scalar_tensor_tensor(
                        xc[:], t[:], float(r_b), xk,
                        mybir.AluOpType.mult, mybir.AluOpType.add,
                    )
                    xcs.append(xc)

                ost = osp.tile([PCW, PC, BNB], F32, tag="ost", name=f"ost{bb}")
                for pc in range(PC):
                    pss = [
                        psp.tile([PCW, NTW], F32, tag="ps", name=f"ps{bb}_{pc}_{nt}")
                        for nt in range(NT)
                    ]
                    for k in range(LC):
                        for nt in range(NT):
                            nc.tensor.matmul(
                                pss[nt][:],
                                wts[k][:, pc * PCW:(pc + 1) * PCW],
                                xcs[k][:, nt * NTW:(nt + 1) * NTW],
                                start=(k == 0),
                                stop=(k == LC - 1),
                            )
                    for nt in range(NT):
                        nc.scalar.activation(
                            ost[:, pc, nt * NTW:(nt + 1) * NTW],
                            pss[nt][:],
                            mybir.ActivationFunctionType.Identity,
                            bias=btile[:, pc:pc + 1],
                        )
                nc.sync.dma_start(o_d[bb], ost[:])

    nc.compile()
    return nc


def kernel(x, Ws, bs, Wt, bt, Wf, bf):
    global LAST_RESULT
    # ---- host-side weight folding (f64, weights only) ----
    A = _movavg_matrix()
    Weff = Ws.astype(np.float64) + (Wt.astype(np.float64) - Ws.astype(np.float64)) @ A
    wf = Wf[0].astype(np.float64)                      # (3,)
    order = np.argsort(np.abs(wf))                     # ascending |wf|
    ca, cb, cc = int(order[0]), int(order[1]), int(order[2])
    r_a = float(wf[ca] / wf[cb]) if wf[cb] != 0 else 0.0
    r_b = float(wf[cb] / wf[cc]) if wf[cc] != 0 else 0.0
    s = float(wf[cc])
    Wp = (s * Weff) if s != 0 else Weff * 0.0          # (336, 512)
    WT = np.ascontiguousarray(Wp.T).reshape(LC, 128, P).astype(ml_dtypes.bfloat16)
    bias = wf.sum() * (bs.astype(np.float64) + bt.astype(np.float64)) + float(bf[0])
    bias_r = np.ascontiguousarray(bias.astype(np.float32).reshape(PC, PCW).T)

    # ---- build / compile (cached per (r_a, r_b)) ----
    key = (round(r_a, 12), round(r_b, 12))
    if key not in _CACHE:
        _CACHE[key] = _build(r_a, r_b)
    nc = _CACHE[key]

    # ---- host-side sharding / layout (pure data movement) ----
    xb16 = x.astype(ml_dtypes.bfloat16)                # (64,3,512,512)
    xr = xb16.reshape(N_CORES, BPC, C, N, L)
    xr = xr.transpose(0, 2, 4, 1, 3)                   # [core, c, l, bl, n]
    xr = xr[:, (ca, cb, cc)]                           # channel order by |wf|
    xr = xr.reshape(N_CORES, C, LC, 128, BB, BNB)
    xr = xr.transpose(0, 4, 2, 3, 1, 5)                # [core, bb, lc, 128, c, bn]
    xr = xr.reshape(N_CORES, BB, LC, 128, C * BNB)

    in_maps = []
    for i in range(N_CORES):
        in_maps.append({
            "x": np.ascontiguousarray(xr[i]),
            "w": WT,
            "bias": bias_r,
        })

    res = run_bass_kernel_spmd(nc, in_maps, core_ids=list(range(N_CORES)))
    LAST_RESULT = res

    # ---- gather / unshard ----
    outs = []
    for i in range(N_CORES):
        o = res.results[i]["o"]                        # (4, 112, 3072) f32
        o = o.reshape(BB, PCW, PC, BNB).transpose(0, 3, 2, 1).reshape(BPC, N, P)
        outs.append(o)
    out = np.stack(outs).reshape(B, N, P)[:, None]     # (64, 1, 512, 336)
    return out.astype(np.float32)
